# revision 18
# baseline (speedup 1.0000x reference)
"""Trainium2 Bass kernel for nn_DenseContrastive (dense contrastive loss).

Math (per the fused reference):
    A = anchors (N, c), E = ema features (N, c), N = 12800, c = 64
    pos_i   = (A_i . E_i) / TEMP
    neg_ij  = (A_i . E_j) / TEMP
    full_i  = [pos_i, neg_i0 .. neg_i(N-1)]          (N+1 entries)
    m_i     = max(full_i)
    denom_i = sum_j exp(full_ij - m_i)
    loss_i  = -log(exp(pos_i - m_i) / (denom_i + EPS) + EPS)
    out     = mean_i loss_i

Key structural fact: with L_i = logsumexp(full_i), the per-row loss is
-log(r_i + EPS) with r_i = exp(pos_i - L_i) <= 1.  Whenever
pos_i <= L_i - G (G ~ 30 logits), r_i <= e^-G << EPS and the fp32 loss
saturates at exactly -log(EPS) = 18.420681.  For the given data the gap
L_i - pos_i is ~300 logits for all but a few hundred rows, so the bulk
of the N^2 work only needs to CERTIFY the gap, not evaluate it.

Certification: m_hat_i = max_{j in S} x_ij over a strided column
subsample S (|S| = KS) is a LOWER bound on L_i.  Rows whose bound does
not clear pos_i + G are recomputed EXACTLY on the host (a few hundred
rows, well under 1 GFLOP) -- the algorithm is input-adaptive but never
wrong; an adversarial input only shifts work to the host fallback, it
cannot produce an incorrect loss.

Sharding: N anchor rows split across 8 cores (1600 each); the sampled
E columns (bf16, channels-on-partitions) replicated per core.  pos is
computed on the host in exact fp32 (0.8 MFLOP).  Per 128-row tile the
PE computes the sampled logit block into PSUM; the DVE max-reduces
four tiles per instruction (quad) to amortize its fixed overhead.
Dummy matmuls during the input-DMA window ramp the PE out of its
throttled power state before real work arrives.
"""

import sys

for _p in ("/opt/trn_rl_repo",):
    if _p not in sys.path:
        sys.path.insert(0, _p)

import numpy as np

import concourse.bass as bass
import concourse.bacc as bacc
import concourse.tile as tile
from concourse import mybir

TEMP = 0.1
EPS = 1e-8
B, C, H, W = 2, 64, 80, 80
N = B * H * W            # 12800 anchors
NCORES = 8
R = N // NCORES          # 1600 rows per core
BLK = 512

KS = 192                 # sampled columns (strided over N), all max-reduced
A0 = 640                 # leading A-shard chunk packed into the head DMA
NWARM = 10               # dummy matmuls to ramp the PE during the DMA wait
BLKW = 128               # warmup matmul free size

KEEP_GAP = 35.0          # certify saturation when 10*(m_hat - pos) >= this
LOSS_FLOOR = -np.log(np.float64(EPS))  # 18.420680743952367

F32 = mybir.dt.float32
BF16 = mybir.dt.bfloat16

# 1600 rows -> 12 full 128-row tiles + one 64-row tile
ROW_TILES = [(i * 128, 128) for i in range(12)] + [(1536, 64)]
NRT = len(ROW_TILES)
GROUPS = [(0, 1, 2, 3), (4, 5, 6, 7), (8, 9, 10, 11, 12)]


def _build() -> bass.Bass:
    nc = bacc.Bacc("TRN2", target_bir_lowering=False)
    head = nc.declare_dram_parameter("head", [C, KS + A0], BF16, isOutput=False)
    a1 = nc.declare_dram_parameter("a1", [C, R - A0], BF16, isOutput=False)
    outp = nc.declare_dram_parameter("outp", [128, NRT], F32, isOutput=True)

    op_max = mybir.AluOpType.max

    with tile.TileContext(nc) as tc:
        with (
            tc.tile_pool(name="res", bufs=1) as res,
            tc.tile_pool(name="psD", bufs=3, space="PSUM") as psD,
            tc.tile_pool(name="psW", bufs=1, space="PSUM") as psW,
        ):
            head_sb = res.tile([C, KS + A0], BF16)  # es cols ++ first A-chunk
            att1_sb = res.tile([C, R - A0], BF16)
            mx_sb = res.tile([128, NRT], F32)       # sampled maxes per tile
            warm_sb = res.tile([C, BLKW], BF16)
            nc.vector.memset(mx_sb[:], 0.0)
            nc.vector.memset(warm_sb[:], 0.0)

            # input DMAs: head on the HWDGE (SP) queue, the A-shard
            # remainder via SWDGE on the otherwise idle Pool engine
            nc.sync.dma_start(out=head_sb[:], in_=head[:])
            nc.gpsimd.dma_start(out=att1_sb[:], in_=a1[:])

            # ramp the PE to full clock while the DMAs land
            psw = psW.tile([128, BLKW], F32, tag="psw")
            for _ in range(NWARM):
                nc.tensor.matmul(
                    out=psw[:, :],
                    lhsT=warm_sb[:, :],
                    rhs=warm_sb[:, :],
                    start=True,
                    stop=True,
                )

            def _att(rt):
                r0, p = ROW_TILES[rt]
                if r0 + p <= A0:
                    return head_sb[:, KS + r0 : KS + r0 + p], p
                return att1_sb[:, r0 - A0 : r0 - A0 + p], p

            for grp in GROUPS:
                nq = len(grp)
                psd = psD.tile([128, 5 * KS], F32, tag="psd")
                for k, rt in enumerate(grp):
                    atT, p = _att(rt)
                    nc.tensor.matmul(
                        out=psd[:p, k * KS : (k + 1) * KS],
                        lhsT=atT,
                        rhs=head_sb[:, :KS],
                        start=True,
                        stop=True,
                    )
                p = ROW_TILES[grp[0]][1]
                nc.vector.tensor_reduce(
                    out=mx_sb[:p, grp[0] : grp[0] + nq],
                    in_=psd[:p, : nq * KS].rearrange("p (b x) -> p b x", b=nq),
                    axis=mybir.AxisListType.X,
                    op=op_max,
                )

            nc.sync.dma_start(out=outp[:], in_=mx_sb[:])

    if not nc.is_finalized():
        nc.finalize()
    return nc


_NC_CACHE: list = []


def _get_nc() -> bass.Bass:
    if not _NC_CACHE:
        _NC_CACHE.append(_build())
    return _NC_CACHE[0]


_RUNNER_CACHE: list = []


def _get_runner():
    """Build the sharded PJRT executable once and reuse it across calls.

    Mirrors bass2jax.run_bass_via_pjrt's multi-core branch, with the
    jitted callable cached so repeat kernel() calls skip retracing.
    """
    if _RUNNER_CACHE:
        return _RUNNER_CACHE[0]

    import jax
    import numpy as _np
    from jax.sharding import Mesh, PartitionSpec
    from jax.experimental.shard_map import shard_map
    from concourse import mybir as _mybir
    from concourse.bass2jax import (
        _bass_exec_p,
        install_neuronx_cc_hook,
        partition_id_tensor,
    )

    nc = _get_nc()
    install_neuronx_cc_hook()
    partition_name = nc.partition_id_tensor.name if nc.partition_id_tensor else None

    in_names, out_names, out_avals, zero_outs = [], [], [], []
    for alloc in nc.m.functions[0].allocations:
        if not isinstance(alloc, _mybir.MemoryLocationSet):
            continue
        name = alloc.memorylocations[0].name
        if alloc.kind == "ExternalInput":
            if name != partition_name:
                in_names.append(name)
        elif alloc.kind == "ExternalOutput":
            shape = tuple(alloc.tensor_shape)
            dtype = _mybir.dt.np(alloc.dtype)
            out_names.append(name)
            out_avals.append(jax.core.ShapedArray(shape, dtype))
            zero_outs.append(_np.zeros(shape, dtype))
    n_params = len(in_names)
    n_outs = len(out_avals)
    all_in_names = list(in_names) + list(out_names)
    if partition_name is not None:
        all_in_names.append(partition_name)

    def _body(*args):
        operands = list(args)
        if partition_name is not None:
            operands.append(partition_id_tensor())
        outs = _bass_exec_p.bind(
            *operands,
            out_avals=tuple(out_avals),
            in_names=tuple(all_in_names),
            out_names=tuple(out_names),
            lowering_input_output_aliases=(),
            sim_require_finite=False,
            sim_require_nnan=False,
            nc=nc,
        )
        return tuple(outs)

    devices = jax.devices()[:NCORES]
    mesh = Mesh(_np.asarray(devices), ("core",))
    spec_of = {
        "head": PartitionSpec("core"),
        "a1": PartitionSpec("core"),
    }
    in_specs = tuple(spec_of[nm] for nm in in_names) + (
        PartitionSpec("core"),
    ) * n_outs
    out_specs = (PartitionSpec("core"),) * n_outs
    donate = tuple(range(n_params, n_params + n_outs))
    sharded = jax.jit(
        shard_map(
            _body, mesh=mesh, in_specs=in_specs, out_specs=out_specs, check_rep=False
        ),
        donate_argnums=donate,
        keep_unused=True,
    )

    state = (sharded, in_names, out_names, out_avals, zero_outs)
    _RUNNER_CACHE.append(state)
    return state


def _to_bf16(x: np.ndarray) -> np.ndarray:
    try:
        import ml_dtypes

        return x.astype(ml_dtypes.bfloat16)
    except ImportError:
        xi = x.astype(np.float32).view(np.uint32)
        r = (xi >> 16) & 1
        xi = (xi + 0x7FFF + r) & 0xFFFF0000
        return xi.view(np.float32)  # bf16 values in fp32 storage (fallback)


def _sample_indices() -> np.ndarray:
    """KS strided column indices over the N ema features."""
    return (np.arange(KS, dtype=np.int64) * N) // KS


def _prep(proj_main, proj_ema):
    """Shared host-side prep: layouts, pos, per-core feeds."""
    pm = np.ascontiguousarray(np.asarray(proj_main, dtype=np.float32))
    pe = np.ascontiguousarray(np.asarray(proj_ema, dtype=np.float32))
    # (b, c, H, W) -> (c, b*H*W): channels on partitions, anchors on free
    at_full = np.ascontiguousarray(pm.transpose(1, 0, 2, 3).reshape(C, N))
    et_full = np.ascontiguousarray(pe.transpose(1, 0, 2, 3).reshape(C, N))
    pos = (at_full * et_full).sum(axis=0, dtype=np.float32)  # (N,) raw dots

    at_b = _to_bf16(at_full)
    et_b = _to_bf16(et_full)
    es_all = et_b[:, _sample_indices()]

    feeds = []
    for core in range(NCORES):
        sl0 = slice(core * R, core * R + A0)
        sl1 = slice(core * R + A0, (core + 1) * R)
        feeds.append(
            {
                "head": np.ascontiguousarray(
                    np.concatenate([es_all, at_b[:, sl0]], axis=1)
                ),
                "a1": np.ascontiguousarray(at_b[:, sl1]),
            }
        )
    return at_full, et_full, pos, feeds


def _make_core_feeds(proj_main, proj_ema):
    """Per-core input dicts keyed by the kernel's DRAM parameter names
    (used by the trace harness, mirroring kernel() exactly)."""
    return _prep(proj_main, proj_ema)[3]


def _finish(at_full, et_full, pos, mx):
    """Certify floored rows from the device bound, exact-fix the rest.

    mx: (N,) sampled maxes (raw logit units)
    """
    pos_s = 10.0 * pos.astype(np.float64)
    gap = 10.0 * mx.astype(np.float64) - pos_s

    flagged = ~(gap >= KEEP_GAP)                   # NaN-safe: NaN -> flagged
    loss = np.full(N, LOSS_FLOOR, dtype=np.float64)
    if flagged.any():
        f = np.nonzero(flagged)[0]
        a64 = at_full.T[f].astype(np.float64)      # (F, C)
        e64 = et_full.astype(np.float64)           # (C, N)
        x = (a64 @ e64) / TEMP                     # (F, N) exact logits
        pf = pos_s[f]
        m = np.maximum(x.max(axis=1), pf)
        denom = np.exp(x - m[:, None]).sum(axis=1) + np.exp(pf - m)
        r = np.exp(pf - m) / (denom + EPS)
        loss[f] = -np.log(r + EPS)
    return np.float32(loss.mean())


def kernel(proj_main, proj_ema, label_main, label_ema, patch_num):
    # labels / patch_num never influence the loss; only the projections do.
    at_full, et_full, pos, feeds = _prep(proj_main, proj_ema)

    sharded, in_names, out_names, out_avals, zero_outs = _get_runner()
    stacked = {
        nm: np.ascontiguousarray(np.concatenate([f[nm] for f in feeds], axis=0))
        for nm in in_names
    }
    args = [stacked[nm] for nm in in_names]
    args += [
        np.zeros((NCORES * z.shape[0], *z.shape[1:]), z.dtype) for z in zero_outs
    ]
    out_arrs = sharded(*args)
    outp = np.asarray(out_arrs[out_names.index("outp")])  # (8*128, NRT)

    mx = (
        outp.reshape(NCORES, 128, NRT)
        .transpose(0, 2, 1)
        .reshape(NCORES, NRT * 128)[:, :R]
        .reshape(N)
    )
    return _finish(at_full, et_full, pos, mx)


if __name__ == "__main__":
    _build()
    print("build OK")


# revision 20
# speedup vs baseline: 1.1178x; 1.1178x over previous
"""Trainium2 Bass kernel for nn_DenseContrastive (dense contrastive loss).

Math (per the fused reference):
    A = anchors (N, c), E = ema features (N, c), N = 12800, c = 64
    pos_i   = (A_i . E_i) / TEMP
    neg_ij  = (A_i . E_j) / TEMP
    full_i  = [pos_i, neg_i0 .. neg_i(N-1)]          (N+1 entries)
    m_i     = max(full_i)
    denom_i = sum_j exp(full_ij - m_i)
    loss_i  = -log(exp(pos_i - m_i) / (denom_i + EPS) + EPS)
    out     = mean_i loss_i

Key structural fact: with L_i = logsumexp(full_i), the per-row loss is
-log(r_i + EPS) with r_i = exp(pos_i - L_i) <= 1.  Whenever
pos_i <= L_i - G (G ~ 30 logits), r_i <= e^-G << EPS and the fp32 loss
saturates at exactly -log(EPS) = 18.420681.  For the given data the gap
L_i - pos_i is ~300 logits for all but a few hundred rows, so the bulk
of the N^2 work only needs to CERTIFY the gap, not evaluate it.

Certification: m_hat_i = max_{j in S} x_ij over a strided column
subsample S (|S| = KS) is a LOWER bound on L_i.  Rows whose bound does
not clear pos_i + G are recomputed EXACTLY on the host (a few hundred
rows, well under 1 GFLOP) -- the algorithm is input-adaptive but never
wrong; an adversarial input only shifts work to the host fallback, it
cannot produce an incorrect loss.

Sharding: N anchor rows split across 8 cores (1600 each); the sampled
E columns (bf16, channels-on-partitions) replicated per core.  pos is
computed on the host in exact fp32 (0.8 MFLOP).  Per 128-row tile the
PE computes the sampled logit block into PSUM; the DVE max-reduces
four tiles per instruction (quad) to amortize its fixed overhead.
Dummy matmuls during the input-DMA window ramp the PE out of its
throttled power state before real work arrives.
"""

import sys

for _p in ("/opt/trn_rl_repo",):
    if _p not in sys.path:
        sys.path.insert(0, _p)

import numpy as np

import concourse.bass as bass
import concourse.bacc as bacc
import concourse.tile as tile
from concourse import mybir

TEMP = 0.1
EPS = 1e-8
B, C, H, W = 2, 64, 80, 80
N = B * H * W            # 12800 anchors
NCORES = 8
R = N // NCORES          # 1600 rows per core
BLK = 512

KS = 128                 # sampled columns (strided over N), all max-reduced
A0 = 640                 # leading A-shard chunk packed into the head DMA
NWARM = 10               # dummy matmuls to ramp the PE during the DMA wait
BLKW = 128               # warmup matmul free size

KEEP_GAP = 35.0          # certify saturation when 10*(m_hat - pos) >= this
LOSS_FLOOR = -np.log(np.float64(EPS))  # 18.420680743952367

F32 = mybir.dt.float32
BF16 = mybir.dt.bfloat16

# 1600 rows -> 12 full 128-row tiles + one 64-row tile
ROW_TILES = [(i * 128, 128) for i in range(12)] + [(1536, 64)]
NRT = len(ROW_TILES)
GROUPS = [(0, 1, 2, 3), (4, 5, 6, 7), (8, 9, 10, 11, 12)]


def _build() -> bass.Bass:
    nc = bacc.Bacc("TRN2", target_bir_lowering=False)
    head = nc.declare_dram_parameter("head", [C, KS + A0], BF16, isOutput=False)
    a1 = nc.declare_dram_parameter("a1", [C, R - A0], BF16, isOutput=False)
    outp = nc.declare_dram_parameter("outp", [128, NRT], F32, isOutput=True)

    op_max = mybir.AluOpType.max

    with tile.TileContext(nc) as tc:
        with (
            tc.tile_pool(name="res", bufs=1) as res,
            tc.tile_pool(name="psD", bufs=3, space="PSUM") as psD,
            tc.tile_pool(name="psW", bufs=1, space="PSUM") as psW,
        ):
            head_sb = res.tile([C, KS + A0], BF16)  # es cols ++ first A-chunk
            att1_sb = res.tile([C, R - A0], BF16)
            mx_sb = res.tile([128, NRT], F32)       # sampled maxes per tile
            warm_sb = res.tile([C, BLKW], BF16)
            nc.vector.memset(mx_sb[:], 0.0)
            nc.vector.memset(warm_sb[:], 0.0)

            # input DMAs: head on the HWDGE (SP) queue, the A-shard
            # remainder via SWDGE on the otherwise idle Pool engine
            nc.sync.dma_start(out=head_sb[:], in_=head[:])
            nc.gpsimd.dma_start(out=att1_sb[:], in_=a1[:])

            # ramp the PE to full clock while the DMAs land
            psw = psW.tile([128, BLKW], F32, tag="psw")
            for _ in range(NWARM):
                nc.tensor.matmul(
                    out=psw[:, :],
                    lhsT=warm_sb[:, :],
                    rhs=warm_sb[:, :],
                    start=True,
                    stop=True,
                )

            def _att(rt):
                r0, p = ROW_TILES[rt]
                if r0 + p <= A0:
                    return head_sb[:, KS + r0 : KS + r0 + p], p
                return att1_sb[:, r0 - A0 : r0 - A0 + p], p

            for grp in GROUPS:
                nq = len(grp)
                psd = psD.tile([128, 5 * KS], F32, tag="psd")
                for k, rt in enumerate(grp):
                    atT, p = _att(rt)
                    nc.tensor.matmul(
                        out=psd[:p, k * KS : (k + 1) * KS],
                        lhsT=atT,
                        rhs=head_sb[:, :KS],
                        start=True,
                        stop=True,
                    )
                p = ROW_TILES[grp[0]][1]
                nc.vector.tensor_reduce(
                    out=mx_sb[:p, grp[0] : grp[0] + nq],
                    in_=psd[:p, : nq * KS].rearrange("p (b x) -> p b x", b=nq),
                    axis=mybir.AxisListType.X,
                    op=op_max,
                )

            nc.sync.dma_start(out=outp[:], in_=mx_sb[:])

    if not nc.is_finalized():
        nc.finalize()
    return nc


_NC_CACHE: list = []


def _get_nc() -> bass.Bass:
    if not _NC_CACHE:
        _NC_CACHE.append(_build())
    return _NC_CACHE[0]


_RUNNER_CACHE: list = []


def _get_runner():
    """Build the sharded PJRT executable once and reuse it across calls.

    Mirrors bass2jax.run_bass_via_pjrt's multi-core branch, with the
    jitted callable cached so repeat kernel() calls skip retracing.
    """
    if _RUNNER_CACHE:
        return _RUNNER_CACHE[0]

    import jax
    import numpy as _np
    from jax.sharding import Mesh, PartitionSpec
    from jax.experimental.shard_map import shard_map
    from concourse import mybir as _mybir
    from concourse.bass2jax import (
        _bass_exec_p,
        install_neuronx_cc_hook,
        partition_id_tensor,
    )

    nc = _get_nc()
    install_neuronx_cc_hook()
    partition_name = nc.partition_id_tensor.name if nc.partition_id_tensor else None

    in_names, out_names, out_avals, zero_outs = [], [], [], []
    for alloc in nc.m.functions[0].allocations:
        if not isinstance(alloc, _mybir.MemoryLocationSet):
            continue
        name = alloc.memorylocations[0].name
        if alloc.kind == "ExternalInput":
            if name != partition_name:
                in_names.append(name)
        elif alloc.kind == "ExternalOutput":
            shape = tuple(alloc.tensor_shape)
            dtype = _mybir.dt.np(alloc.dtype)
            out_names.append(name)
            out_avals.append(jax.core.ShapedArray(shape, dtype))
            zero_outs.append(_np.zeros(shape, dtype))
    n_params = len(in_names)
    n_outs = len(out_avals)
    all_in_names = list(in_names) + list(out_names)
    if partition_name is not None:
        all_in_names.append(partition_name)

    def _body(*args):
        operands = list(args)
        if partition_name is not None:
            operands.append(partition_id_tensor())
        outs = _bass_exec_p.bind(
            *operands,
            out_avals=tuple(out_avals),
            in_names=tuple(all_in_names),
            out_names=tuple(out_names),
            lowering_input_output_aliases=(),
            sim_require_finite=False,
            sim_require_nnan=False,
            nc=nc,
        )
        return tuple(outs)

    devices = jax.devices()[:NCORES]
    mesh = Mesh(_np.asarray(devices), ("core",))
    spec_of = {
        "head": PartitionSpec("core"),
        "a1": PartitionSpec("core"),
    }
    in_specs = tuple(spec_of[nm] for nm in in_names) + (
        PartitionSpec("core"),
    ) * n_outs
    out_specs = (PartitionSpec("core"),) * n_outs
    donate = tuple(range(n_params, n_params + n_outs))
    sharded = jax.jit(
        shard_map(
            _body, mesh=mesh, in_specs=in_specs, out_specs=out_specs, check_rep=False
        ),
        donate_argnums=donate,
        keep_unused=True,
    )

    state = (sharded, in_names, out_names, out_avals, zero_outs)
    _RUNNER_CACHE.append(state)
    return state


def _to_bf16(x: np.ndarray) -> np.ndarray:
    try:
        import ml_dtypes

        return x.astype(ml_dtypes.bfloat16)
    except ImportError:
        xi = x.astype(np.float32).view(np.uint32)
        r = (xi >> 16) & 1
        xi = (xi + 0x7FFF + r) & 0xFFFF0000
        return xi.view(np.float32)  # bf16 values in fp32 storage (fallback)


def _sample_indices() -> np.ndarray:
    """KS strided column indices over the N ema features."""
    return (np.arange(KS, dtype=np.int64) * N) // KS


def _prep(proj_main, proj_ema):
    """Shared host-side prep: layouts, pos, per-core feeds."""
    pm = np.ascontiguousarray(np.asarray(proj_main, dtype=np.float32))
    pe = np.ascontiguousarray(np.asarray(proj_ema, dtype=np.float32))
    # (b, c, H, W) -> (c, b*H*W): channels on partitions, anchors on free
    at_full = np.ascontiguousarray(pm.transpose(1, 0, 2, 3).reshape(C, N))
    et_full = np.ascontiguousarray(pe.transpose(1, 0, 2, 3).reshape(C, N))
    pos = (at_full * et_full).sum(axis=0, dtype=np.float32)  # (N,) raw dots

    at_b = _to_bf16(at_full)
    et_b = _to_bf16(et_full)
    es_all = et_b[:, _sample_indices()]

    feeds = []
    for core in range(NCORES):
        sl0 = slice(core * R, core * R + A0)
        sl1 = slice(core * R + A0, (core + 1) * R)
        feeds.append(
            {
                "head": np.ascontiguousarray(
                    np.concatenate([es_all, at_b[:, sl0]], axis=1)
                ),
                "a1": np.ascontiguousarray(at_b[:, sl1]),
            }
        )
    return at_full, et_full, pos, feeds


def _make_core_feeds(proj_main, proj_ema):
    """Per-core input dicts keyed by the kernel's DRAM parameter names
    (used by the trace harness, mirroring kernel() exactly)."""
    return _prep(proj_main, proj_ema)[3]


def _finish(at_full, et_full, pos, mx):
    """Certify floored rows from the device bound, exact-fix the rest.

    mx: (N,) sampled maxes (raw logit units)
    """
    pos_s = 10.0 * pos.astype(np.float64)
    gap = 10.0 * mx.astype(np.float64) - pos_s

    flagged = ~(gap >= KEEP_GAP)                   # NaN-safe: NaN -> flagged
    loss = np.full(N, LOSS_FLOOR, dtype=np.float64)
    if flagged.any():
        f = np.nonzero(flagged)[0]
        e64 = et_full.astype(np.float64)           # (C, N)
        for c0 in range(0, len(f), 2048):          # bound peak host memory
            fc = f[c0 : c0 + 2048]
            a64 = at_full.T[fc].astype(np.float64)     # (F, C)
            x = (a64 @ e64) / TEMP                     # (F, N) exact logits
            pf = pos_s[fc]
            m = np.maximum(x.max(axis=1), pf)
            denom = np.exp(x - m[:, None]).sum(axis=1) + np.exp(pf - m)
            r = np.exp(pf - m) / (denom + EPS)
            loss[fc] = -np.log(r + EPS)
    return np.float32(loss.mean())


def kernel(proj_main, proj_ema, label_main, label_ema, patch_num):
    # labels / patch_num never influence the loss; only the projections do.
    at_full, et_full, pos, feeds = _prep(proj_main, proj_ema)

    sharded, in_names, out_names, out_avals, zero_outs = _get_runner()
    stacked = {
        nm: np.ascontiguousarray(np.concatenate([f[nm] for f in feeds], axis=0))
        for nm in in_names
    }
    args = [stacked[nm] for nm in in_names]
    args += [
        np.zeros((NCORES * z.shape[0], *z.shape[1:]), z.dtype) for z in zero_outs
    ]
    out_arrs = sharded(*args)
    outp = np.asarray(out_arrs[out_names.index("outp")])  # (8*128, NRT)

    mx = (
        outp.reshape(NCORES, 128, NRT)
        .transpose(0, 2, 1)
        .reshape(NCORES, NRT * 128)[:, :R]
        .reshape(N)
    )
    return _finish(at_full, et_full, pos, mx)


if __name__ == "__main__":
    _build()
    print("build OK")
